# revision 18
# baseline (speedup 1.0000x reference)
"""GroupedQueryAttention Trainium2 kernel (8-core SPMD), v2.

Reference op: RMSNorm -> in-proj (q/k/v) -> RoPE -> causal GQA attention
-> out-proj -> residual.  b=2, s=2048, d_model=2048, 32 q-heads / 8 KV
groups, head dim 64, fp32.

Sharding: core c handles batch b = c//4 and KV groups (2j, 2j+1), j = c%4
(data parallel over batch x tensor parallel over KV groups, Megatron
style).  Each core computes the full in-projection restricted to its 8
heads' channels, attention for its 8 heads, and a partial out-projection
(row-parallel).  The host sums the 4 partials per batch and adds the
residual.

v2 changes vs v1 (993us):
  * qk PSUM double-buffered (2 tags x 2 banks) so QK(t+1) overlaps
    exp(t); PSUM = acc(ip/ss/op shared, 2) + qk(4) + av(2) = 8 banks.
  * inv_rms and softmax denominators via reciprocal_approx_fast on the
    natural [1/2, 512] rows - kills the DRAM transpose bounces and the
    ACT denominator copies.
  * All DVE traffic 16-bit where possible (cos/sin/tri tables f16) for
    DVE 2x mode; xsq moved to ACT (Square), V inv_rms scale folded as
    one TT instead of per-tile ACT scale-copies.
  * w_out and all of x resident in SBUF (no per-chunk weight reloads).
  * yT output f16 (halves store traffic; host accumulates in fp32).
"""

import numpy as np
from contextlib import ExitStack

import concourse.bass as bass
from concourse import bacc as _bacc
import concourse.mybir as mybir
import concourse.tile as tile
from concourse.bass import ts

import os
f32 = mybir.dt.float32
f32r = mybir.dt.float32r
f16 = mybir.dt.float16
MDT = {"f32r": f32r, "f16": f16, "bf16": mybir.dt.bfloat16}[os.environ.get("GQA_MM_DT", "f16")]
try:
    import ml_dtypes
    _BF16_NP = ml_dtypes.bfloat16
except ImportError:
    _BF16_NP = None
MDT_NP = {f32r: np.float32, f16: np.float16, mybir.dt.bfloat16: _BF16_NP}[MDT]
AF = mybir.ActivationFunctionType
ALU = mybir.AluOpType

D = 2048          # model dim
CH = 768          # per-core in-proj channels (8 q heads + 2 k + 2 v)
TOKC = 512        # token chunk
NKT = D // 128    # 16 k-tiles over model dim
RMS_EPS = 1e-6
ROPE_THETA = 10000.0
NCORES = 8


def build_program(S=2048):
    NCH = S // TOKC          # token chunks
    NSK = S // 128           # sk tiles
    nc = _bacc.Bacc(None)

    xT_d = nc.dram_tensor("xT", [D, S], MDT, kind="ExternalInput")
    w_inT_d = nc.dram_tensor("w_inT", [D, CH], MDT, kind="ExternalInput")
    w_outT_d = nc.dram_tensor("w_outT", [512, D], MDT, kind="ExternalInput")
    cos_d = nc.dram_tensor("cos_t", [128, S], MDT, kind="ExternalInput")
    sin_d = nc.dram_tensor("sin_t", [128, S], MDT, kind="ExternalInput")
    msk_d = nc.dram_tensor("mskA", [128, 128], MDT, kind="ExternalInput")
    id_d = nc.dram_tensor("id128", [128, 128], MDT, kind="ExternalInput")
    oner_d = nc.dram_tensor("oner", [1], MDT, kind="ExternalInput")
    eps_d = nc.dram_tensor("epsc", [1], f32, kind="ExternalInput")
    yT_d = nc.dram_tensor("yT", [D, S], MDT, kind="ExternalOutput")

    with tile.TileContext(nc) as tc, ExitStack() as ctx:
        sb = ctx.enter_context(tc.tile_pool(name="sb", bufs=1))
        sbs = ctx.enter_context(tc.tile_pool(name="sbs", bufs=2))
        dramp = ctx.enter_context(tc.tile_pool(name="dram", bufs=1, space="DRAM"))

        # persistent SBUF
        w_in_sb = sb.tile([128, NKT, CH], MDT, name="w_in_sb")
        w_out_sb = sb.tile([128, 4, D], MDT, name="w_out_sb")
        xt_sb = sb.tile([128, NKT, S], MDT, name="xt_sb")
        qkv = sb.tile([128, 6, S], MDT, name="qkv")        # ch tiles 0-3 q pairs, 4 k, 5 v
        oT = sb.tile([128, 4, S], MDT, name="oT")
        vA = sb.tile([128, NSK, 65], MDT, name="vA")       # V + ones col, group 0
        vB = sb.tile([128, NSK, 65], MDT, name="vB")       # group 1
        cos_sb = sb.tile([128, S], MDT, name="cos_sb")
        sin_sb = sb.tile([128, S], MDT, name="sin_sb")
        msk_sb = sb.tile([128, 128], MDT, name="msk_sb")
        id_sb = sb.tile([128, 128], MDT, name="id_sb")
        ones_sb = sb.tile([128, 1], MDT, name="ones_sb")
        eps_sb = sb.tile([1, 1], f32, name="eps_sb")

        nrm_dr = dramp.tile([NCH, TOKC], MDT, name="nrm_dr")
        db_dr = dramp.tile([NCH, 4, 2, TOKC], f32, name="db_dr")

        w_inT_v = w_inT_d.rearrange("(o p) c -> p o c", p=128)
        nc.sync.dma_start(w_in_sb[:], w_inT_v[:])
        w_outT_v = w_outT_d.rearrange("(o p) c -> p o c", p=128)
        nc.sync.dma_start(w_out_sb[:], w_outT_v[:])
        xT_v = xT_d.rearrange("(o p) s -> p o s", p=128)
        nc.sync.dma_start(cos_sb[:], cos_d[:])
        nc.sync.dma_start(sin_sb[:], sin_d[:])
        nc.sync.dma_start(msk_sb[:], msk_d[:])
        nc.sync.dma_start(id_sb[:], id_d[:])
        nc.sync.dma_start(ones_sb[:], oner_d[None, :].to_broadcast((128, 1)))
        nc.sync.dma_start(vA[:, :, 64:65], oner_d[None, None, :].to_broadcast((128, NSK, 1)))
        nc.sync.dma_start(vB[:, :, 64:65], oner_d[None, None, :].to_broadcast((128, NSK, 1)))
        nc.sync.dma_start(eps_sb[:], eps_d[None, :])

        # PSUM: acc (in-proj / sum-sq / out-proj, 1 bank x2) + qk (2 banks
        # x2) + av (1 bank x2) = 8 banks.
        with tc.tile_pool(name="ps", bufs=1, space="PSUM") as ps:

            def emit_prelude(c):
                cs = slice(c * TOKC, (c + 1) * TOKC)
                nc.sync.dma_start(xt_sb[:, :, cs], xT_v[:, :, cs])
                # sum of squares -> 1/rms row, then broadcast
                ss = ps.tile([1, TOKC], f32, tag="acc", bufs=2, name=f"ss_{c}")
                for kt in range(NKT):
                    xsq = sbs.tile([128, TOKC], MDT, tag="xsq", bufs=3,
                                   name=f"xsq_{c}_{kt}")
                    nc.vector.tensor_tensor(xsq[:], xt_sb[:, kt, cs],
                                            xt_sb[:, kt, cs], ALU.mult)
                    nc.tensor.matmul(ss[:], ones_sb[:], xsq[:],
                                     start=(kt == 0), stop=(kt == NKT - 1))
                # 1/rms = exp(-0.5 * ln(ss/D + eps)): ln+exp live in one ACT
                # table set (no sqrt-set thrash against attention exp).
                ln_row = sbs.tile([1, TOKC], f32, tag="ln_row", bufs=2,
                                  name=f"ln_row_{c}")
                nc.scalar.activation(ln_row[:], ss[:], AF.Ln,
                                     bias=eps_sb[:], scale=1.0 / D)
                inv16 = sbs.tile([1, TOKC], MDT, tag="inv16", bufs=2,
                                 name=f"inv16_{c}")
                nc.scalar.activation(inv16[:], ln_row[:], AF.Exp, scale=-0.5)
                nc.sync.dma_start(nrm_dr[c][None, :], inv16[:])
                inv128 = sbs.tile([128, TOKC], MDT, tag="inv128", bufs=2,
                                  name=f"inv128_{c}")
                nc.sync.dma_start(inv128[:],
                                  nrm_dr[c][None, :].to_broadcast((128, TOKC)))
                # rope tables scaled by inv_rms (f16 for DVE 2x mode)
                cosi = sbs.tile([128, TOKC], MDT, tag="cosi", bufs=2,
                                name=f"cosi_{c}")
                nc.vector.tensor_tensor(cosi[:], cos_sb[:, cs], inv128[:], ALU.mult)
                sini = sbs.tile([128, TOKC], MDT, tag="sini", bufs=2,
                                name=f"sini_{c}")
                nc.vector.tensor_tensor(sini[:], sin_sb[:, cs], inv128[:], ALU.mult)
                return cosi, sini, inv128

            def emit_inproj_m(c, m, state):
                cs = slice(c * TOKC, (c + 1) * TOKC)
                cosi, sini, inv128 = state
                ip = ps.tile([128, TOKC], f32, tag="acc", bufs=2,
                             name=f"ip{m}_{c}")
                for kt in range(NKT):
                    nc.tensor.matmul(ip[:], w_in_sb[:, kt, ts(m, 128)],
                                     xt_sb[:, kt, cs],
                                     start=(kt == 0), stop=(kt == NKT - 1))
                nc.vector.tensor_copy(qkv[:, m, cs], ip[:])
                if m < 5:
                    # rope in place, inv_rms folded into the tables.  The
                    # rotate-half partition swap runs as 4 small SBUF->SBUF
                    # DMAs (sin table is pre-rotated host-side), leaving
                    # only 3 full-width DVE ops.
                    rot = sbs.tile([128, TOKC], MDT, tag="rot", bufs=2,
                                   name=f"rot_{c}_{m}")
                    for dst, src in ((0, 32), (32, 0), (64, 96), (96, 64)):
                        nc.sync.dma_start(rot[dst:dst + 32, :],
                                          qkv[src:src + 32, m, cs])
                    nc.vector.tensor_tensor(rot[:], rot[:], sini[:], ALU.mult)
                    nc.vector.tensor_tensor(qkv[:, m, cs], qkv[:, m, cs],
                                            cosi[:], ALU.mult)
                    nc.vector.tensor_tensor(qkv[:, m, cs], qkv[:, m, cs],
                                            rot[:], ALU.add)
                else:
                    # V: fold per-token inv_rms once, then transpose to
                    # [token, dv] tiles
                    nc.vector.tensor_tensor(qkv[:, 5, cs], qkv[:, 5, cs],
                                            inv128[:], ALU.mult)
                    for tl in range(TOKC // 128):
                        t = c * (TOKC // 128) + tl
                        vtt = sbs.tile([128, 128], MDT, tag="vtt", bufs=2,
                                       name=f"vtt_{t}")
                        nc.sync.dma_start(vtt[:], qkv[:, 5, ts(t, 128)],
                                          transpose=True)
                        nc.vector.tensor_copy(vA[:, t, 0:64], vtt[:, 0:64])
                        nc.vector.tensor_copy(vB[:, t, 0:64], vtt[:, 64:128])

            def emit_attn_pair(c, p):
                cs = slice(c * TOKC, (c + 1) * TOKC)
                n_t = 4 * (c + 1)
                avA = ps.tile([65, TOKC], f32, tag="av", bufs=2,
                              name=f"avA_{c}_{p}")
                avB = ps.tile([65, TOKC], f32, tag="av", bufs=2,
                              name=f"avB_{c}_{p}")
                for t in range(n_t):
                    j0 = max(0, t - 4 * c) * 128
                    diag = t >= 4 * c
                    qk = ps.tile([128, 2, TOKC], f32, tag="qk", bufs=2,
                                 name=f"qk_{c}_{p}_{t}")
                    # the pair's two heads: row-tiled concurrent K=64 matmuls
                    nc.tensor.matmul(
                        qk[:, 0, j0:],
                        qkv[0:64, 4, ts(t, 128)],
                        qkv[0:64, p, c * TOKC + j0:(c + 1) * TOKC],
                        start=True, stop=not diag,
                    )
                    nc.tensor.matmul(
                        qk[:, 1, j0:],
                        qkv[64:128, 4, ts(t, 128)],
                        qkv[64:128, p, c * TOKC + j0:(c + 1) * TOKC],
                        start=True, stop=not diag,
                    )
                    if diag:
                        # causal mask for the diagonal 128x128 block as a PE
                        # accumulate: qk[k, j] += -30 * [j < k] (msk^T @ I),
                        # keeping the exp -> av chain off the vector engine.
                        nc.tensor.matmul(qk[:, 0, j0:j0 + 128], msk_sb[:],
                                         id_sb[:], start=False, stop=True)
                        nc.tensor.matmul(qk[:, 1, j0:j0 + 128], msk_sb[:],
                                         id_sb[:], start=False, stop=True)
                    e = sbs.tile([128, 2, TOKC], MDT, tag="e", bufs=4,
                                 name=f"e_{c}_{p}_{t}")
                    nc.scalar.activation(e[:, :, j0:], qk[:, :, j0:], AF.Exp)
                    nc.tensor.matmul(avA[:, j0:], vA[:, t, :], e[:, 0, j0:],
                                     start=(t == 0), stop=(t == n_t - 1))
                    nc.tensor.matmul(avB[:, j0:], vB[:, t, :], e[:, 1, j0:],
                                     start=(t == 0), stop=(t == n_t - 1))
                # Evacuate av immediately (ACT, PSUM-close) so the av banks
                # free up for the next pair without waiting for the
                # denominator DMA round-trip.
                oTu = sbs.tile([128, TOKC], MDT, tag="oTu", bufs=2,
                               name=f"oTu_{c}_{p}")
                nc.scalar.copy(oTu[0:64, :], avA[0:64, :])
                nc.scalar.copy(oTu[64:128, :], avB[0:64, :])
                # softmax denominators: row 64 of each AV psum.  Copy the
                # rows to SBUF partition 0 first: reciprocal_approx_fast
                # (custom DVE op) reads the wrong partition when its PSUM
                # source has a non-zero base partition (HW-verified).
                dinvA = sbs.tile([1, TOKC], f32, tag="dinvA", bufs=2,
                                 name=f"dinvA_{c}_{p}")
                nc.vector.tensor_copy(dinvA[:], avA[64:65, :])
                nc.vector.reciprocal_approx_fast(dinvA[:], dinvA[:])
                dinvB = sbs.tile([1, TOKC], f32, tag="dinvB", bufs=2,
                                 name=f"dinvB_{c}_{p}")
                nc.vector.tensor_copy(dinvB[:], avB[64:65, :])
                nc.vector.reciprocal_approx_fast(dinvB[:], dinvB[:])
                nc.sync.dma_start(db_dr[c, p, 0][None, :], dinvA[:])
                nc.sync.dma_start(db_dr[c, p, 1][None, :], dinvB[:])
                db2 = sbs.tile([128, TOKC], f32, tag="db2", bufs=2,
                               name=f"db2_{c}_{p}")
                nc.sync.dma_start(
                    db2[0:64, :], db_dr[c, p, 0][None, :].to_broadcast((64, TOKC)))
                nc.sync.dma_start(
                    db2[64:128, :], db_dr[c, p, 1][None, :].to_broadcast((64, TOKC)))
                nc.vector.tensor_tensor(oT[0:64, p, cs], oTu[0:64, :],
                                        db2[0:64, :], ALU.mult)
                nc.vector.tensor_tensor(oT[64:128, p, cs], oTu[64:128, :],
                                        db2[64:128, :], ALU.mult)

            def emit_outproj_part(c, ms):
                cs = slice(c * TOKC, (c + 1) * TOKC)
                for m in ms:
                    op = ps.tile([128, TOKC], f32, tag="acc", bufs=2,
                                 name=f"op_{c}_{m}")
                    for kt in range(4):
                        nc.tensor.matmul(op[:], w_out_sb[:, kt, ts(m, 128)],
                                         oT[:, kt, cs],
                                         start=(kt == 0), stop=(kt == 3))
                    yt = sbs.tile([128, TOKC], MDT, tag="yt", bufs=3,
                                  name=f"yt_{c}_{m}")
                    nc.vector.tensor_copy(yt[:], op[:])
                    nc.sync.dma_start(yT_d[ts(m, 128), cs], yt[:])

            for c in range(NCH):
                st = emit_prelude(c)
                for m in range(6):
                    emit_inproj_m(c, m, st)
                    if c > 0 and m < 4:
                        emit_attn_pair(c - 1, m)
                    if c > 0 and m >= 4:
                        emit_outproj_part(c - 1, range(8 * (m - 4), 8 * (m - 3)))
            for p in range(4):
                emit_attn_pair(NCH - 1, p)
            emit_outproj_part(NCH - 1, range(16))

    nc.finalize()
    return nc


# ------------------------------- host side ----------------------------------

def _rope_tables(S):
    inv_freq = ROPE_THETA ** (-np.arange(0, 64, 2, dtype=np.float64) / 64.0)  # [32]
    ang = np.arange(S, dtype=np.float64)[:, None] * inv_freq[None, :]          # [S, 32]
    cosb = np.cos(ang).T.astype(np.float32)   # [32, S]
    sinb = np.sin(ang).T.astype(np.float32)
    cos128 = np.tile(cosb, (4, 1))                                             # [128, S]
    # pre-rotated sign pattern: row block dst reads the sin factor of the
    # block it was swapped with on-device ((0,32),(32,0),(64,96),(96,64))
    sin128 = np.concatenate([-sinb, sinb, -sinb, sinb], axis=0)                # [128, S]
    return np.ascontiguousarray(cos128), np.ascontiguousarray(sin128)


def host_prepare(x, w_in, w_out, rms_w):
    """Build the 8 per-core input maps."""
    S = x.shape[1]
    x = np.asarray(x, dtype=np.float32)
    w_eff = np.asarray(w_in, dtype=np.float32) * np.asarray(rms_w, np.float32)[None, :]
    w_out = np.asarray(w_out, dtype=np.float32)
    cos128, sin128 = _rope_tables(S)
    mskA = np.ascontiguousarray(
        -30.0 * np.triu(np.ones((128, 128), dtype=np.float32), 1))
    id128 = np.eye(128, dtype=np.float32)
    qscale = np.float32(64 ** -0.5)

    in_maps = []
    for core in range(NCORES):
        b, j = divmod(core, 4)
        g0, g1 = 2 * j, 2 * j + 1
        rows = []
        for p in range(4):
            for g in (g0, g1):
                rows.extend(range((g * 4 + p) * 64, (g * 4 + p) * 64 + 64))
        for g in (g0, g1):
            rows.extend(range(2048 + g * 64, 2048 + g * 64 + 64))
        for g in (g0, g1):
            rows.extend(range(2560 + g * 64, 2560 + g * 64 + 64))
        w_slice = w_eff[rows, :].copy()
        w_slice[:512, :] *= qscale
        cols = []
        for p in range(4):
            for g in (g0, g1):
                cols.extend(range((g * 4 + p) * 64, (g * 4 + p) * 64 + 64))
        in_maps.append({
            "xT": np.ascontiguousarray(x[b].T).astype(MDT_NP),
            "w_inT": np.ascontiguousarray(w_slice.T).astype(MDT_NP),
            "w_outT": np.ascontiguousarray(w_out[:, cols].T).astype(MDT_NP),
            "cos_t": cos128.astype(MDT_NP),
            "sin_t": sin128.astype(MDT_NP),
            "mskA": mskA.astype(MDT_NP),
            "id128": id128.astype(MDT_NP),
            "oner": np.ones(1, dtype=MDT_NP),
            "epsc": np.full(1, RMS_EPS, dtype=np.float32),
        })
    return in_maps


def assemble(x, results):
    x = np.asarray(x, dtype=np.float32)
    b0 = (results[0]["yT"].astype(np.float32) + results[1]["yT"].astype(np.float32)
          + results[2]["yT"].astype(np.float32) + results[3]["yT"].astype(np.float32))
    b1 = (results[4]["yT"].astype(np.float32) + results[5]["yT"].astype(np.float32)
          + results[6]["yT"].astype(np.float32) + results[7]["yT"].astype(np.float32))
    out = np.empty_like(x)
    out[0] = x[0] + b0.T
    out[1] = x[1] + b1.T
    return out


_PROGRAMS = {}


def _get_program(S):
    if S not in _PROGRAMS:
        _PROGRAMS[S] = build_program(S)
    return _PROGRAMS[S]


def run(x, w_in, w_out, rms_w, trace=False):
    from concourse.bass_utils import run_bass_kernel_spmd
    nc = _get_program(x.shape[1])
    in_maps = host_prepare(x, w_in, w_out, rms_w)
    res = run_bass_kernel_spmd(nc, in_maps, list(range(NCORES)), trace=trace)
    return assemble(x, res.results), res


def kernel(x, w_in, w_out, rms_w):
    out, _ = run(np.asarray(x), np.asarray(w_in), np.asarray(w_out),
                 np.asarray(rms_w))
    return out


# revision 23
# speedup vs baseline: 1.0319x; 1.0319x over previous
"""GroupedQueryAttention Trainium2 kernel (8-core SPMD), v2.

Reference op: RMSNorm -> in-proj (q/k/v) -> RoPE -> causal GQA attention
-> out-proj -> residual.  b=2, s=2048, d_model=2048, 32 q-heads / 8 KV
groups, head dim 64, fp32.

Sharding: core c handles batch b = c//4 and KV groups (2j, 2j+1), j = c%4
(data parallel over batch x tensor parallel over KV groups, Megatron
style).  Each core computes the full in-projection restricted to its 8
heads' channels, attention for its 8 heads, and a partial out-projection
(row-parallel).  The host sums the 4 partials per batch and adds the
residual.

v2 changes vs v1 (993us):
  * qk PSUM double-buffered (2 tags x 2 banks) so QK(t+1) overlaps
    exp(t); PSUM = acc(ip/ss/op shared, 2) + qk(4) + av(2) = 8 banks.
  * inv_rms and softmax denominators via reciprocal_approx_fast on the
    natural [1/2, 512] rows - kills the DRAM transpose bounces and the
    ACT denominator copies.
  * All DVE traffic 16-bit where possible (cos/sin/tri tables f16) for
    DVE 2x mode; xsq moved to ACT (Square), V inv_rms scale folded as
    one TT instead of per-tile ACT scale-copies.
  * w_out and all of x resident in SBUF (no per-chunk weight reloads).
  * yT output f16 (halves store traffic; host accumulates in fp32).
"""

import numpy as np
from contextlib import ExitStack

import concourse.bass as bass
from concourse import bacc as _bacc
import concourse.mybir as mybir
import concourse.tile as tile
from concourse.bass import ts

import os
f32 = mybir.dt.float32
f32r = mybir.dt.float32r
f16 = mybir.dt.float16
MDT = {"f32r": f32r, "f16": f16, "bf16": mybir.dt.bfloat16}[os.environ.get("GQA_MM_DT", "f16")]
try:
    import ml_dtypes
    _BF16_NP = ml_dtypes.bfloat16
except ImportError:
    _BF16_NP = None
MDT_NP = {f32r: np.float32, f16: np.float16, mybir.dt.bfloat16: _BF16_NP}[MDT]
AF = mybir.ActivationFunctionType
ALU = mybir.AluOpType

D = 2048          # model dim
CH = 768          # per-core in-proj channels (8 q heads + 2 k + 2 v)
TOKC = 512        # token chunk
NKT = D // 128    # 16 k-tiles over model dim
RMS_EPS = 1e-6
ROPE_THETA = 10000.0
NCORES = 8


def build_program(S=2048):
    NCH = S // TOKC          # token chunks
    NSK = S // 128           # sk tiles
    nc = _bacc.Bacc(None)

    xT_d = nc.dram_tensor("xT", [D, S], MDT, kind="ExternalInput")
    w_inT_d = nc.dram_tensor("w_inT", [D, CH], MDT, kind="ExternalInput")
    w_outT_d = nc.dram_tensor("w_outT", [512, D], MDT, kind="ExternalInput")
    cos_d = nc.dram_tensor("cos_t", [128, S], MDT, kind="ExternalInput")
    sin_d = nc.dram_tensor("sin_t", [128, S], MDT, kind="ExternalInput")
    msk_d = nc.dram_tensor("mskA", [128, 128], MDT, kind="ExternalInput")
    id_d = nc.dram_tensor("id128", [128, 128], MDT, kind="ExternalInput")
    oner_d = nc.dram_tensor("oner", [1], MDT, kind="ExternalInput")
    eps_d = nc.dram_tensor("epsc", [1], f32, kind="ExternalInput")
    yT_d = nc.dram_tensor("yT", [D, S], MDT, kind="ExternalOutput")

    with tile.TileContext(nc) as tc, ExitStack() as ctx:
        sb = ctx.enter_context(tc.tile_pool(name="sb", bufs=1))
        sbs = ctx.enter_context(tc.tile_pool(name="sbs", bufs=2))
        dramp = ctx.enter_context(tc.tile_pool(name="dram", bufs=1, space="DRAM"))

        # persistent SBUF
        w_in_sb = sb.tile([128, NKT, CH], MDT, name="w_in_sb")
        w_out_sb = sb.tile([128, 4, D], MDT, name="w_out_sb")
        xt_sb = sb.tile([128, NKT, S], MDT, name="xt_sb")
        qkv = sb.tile([128, 6, S], MDT, name="qkv")        # ch tiles 0-3 q pairs, 4 k, 5 v
        oT = sb.tile([128, 4, S], MDT, name="oT")
        vA = sb.tile([128, NSK, 65], MDT, name="vA")       # V + ones col, group 0
        vB = sb.tile([128, NSK, 65], MDT, name="vB")       # group 1
        cos_sb = sb.tile([128, S], MDT, name="cos_sb")
        sin_sb = sb.tile([128, S], MDT, name="sin_sb")
        msk_sb = sb.tile([128, 128], MDT, name="msk_sb")
        id_sb = sb.tile([128, 128], MDT, name="id_sb")
        ones_sb = sb.tile([128, 1], MDT, name="ones_sb")
        eps_sb = sb.tile([1, 1], f32, name="eps_sb")

        nrm_dr = dramp.tile([NCH, TOKC], MDT, name="nrm_dr")
        db_dr = dramp.tile([NCH, 4, 2, TOKC], f32, name="db_dr")

        w_inT_v = w_inT_d.rearrange("(o p) c -> p o c", p=128)
        nc.sync.dma_start(w_in_sb[:], w_inT_v[:])
        w_outT_v = w_outT_d.rearrange("(o p) c -> p o c", p=128)
        nc.sync.dma_start(w_out_sb[:], w_outT_v[:])
        xT_v = xT_d.rearrange("(o p) s -> p o s", p=128)
        nc.sync.dma_start(cos_sb[:], cos_d[:])
        nc.sync.dma_start(sin_sb[:], sin_d[:])
        nc.sync.dma_start(msk_sb[:], msk_d[:])
        nc.sync.dma_start(id_sb[:], id_d[:])
        nc.sync.dma_start(ones_sb[:], oner_d[None, :].to_broadcast((128, 1)))
        nc.sync.dma_start(vA[:, :, 64:65], oner_d[None, None, :].to_broadcast((128, NSK, 1)))
        nc.sync.dma_start(vB[:, :, 64:65], oner_d[None, None, :].to_broadcast((128, NSK, 1)))
        nc.sync.dma_start(eps_sb[:], eps_d[None, :])

        # PSUM: acc (in-proj / sum-sq / out-proj, 1 bank x2) + qk (2 banks
        # x2) + av (1 bank x2) = 8 banks.
        with tc.tile_pool(name="ps", bufs=1, space="PSUM") as ps:

            def emit_prelude(c):
                cs = slice(c * TOKC, (c + 1) * TOKC)
                nc.sync.dma_start(xt_sb[:, :, cs], xT_v[:, :, cs])
                # sum of squares -> 1/rms row, then broadcast
                ss = ps.tile([1, TOKC], f32, tag="acc", bufs=2, name=f"ss_{c}")
                for kt in range(NKT):
                    xsq = sbs.tile([128, TOKC], MDT, tag="xsq", bufs=3,
                                   name=f"xsq_{c}_{kt}")
                    nc.vector.tensor_tensor(xsq[:], xt_sb[:, kt, cs],
                                            xt_sb[:, kt, cs], ALU.mult)
                    nc.tensor.matmul(ss[:], ones_sb[:], xsq[:],
                                     start=(kt == 0), stop=(kt == NKT - 1))
                # 1/rms = exp(-0.5 * ln(ss/D + eps)): ln+exp live in one ACT
                # table set (no sqrt-set thrash against attention exp).
                ln_row = sbs.tile([1, TOKC], f32, tag="ln_row", bufs=2,
                                  name=f"ln_row_{c}")
                nc.scalar.activation(ln_row[:], ss[:], AF.Ln,
                                     bias=eps_sb[:], scale=1.0 / D)
                inv16 = sbs.tile([1, TOKC], MDT, tag="inv16", bufs=2,
                                 name=f"inv16_{c}")
                nc.scalar.activation(inv16[:], ln_row[:], AF.Exp, scale=-0.5)
                nc.sync.dma_start(nrm_dr[c][None, :], inv16[:])
                inv128 = sbs.tile([128, TOKC], MDT, tag="inv128", bufs=2,
                                  name=f"inv128_{c}")
                nc.sync.dma_start(inv128[:],
                                  nrm_dr[c][None, :].to_broadcast((128, TOKC)))
                # rope tables scaled by inv_rms (f16 for DVE 2x mode)
                cosi = sbs.tile([128, TOKC], MDT, tag="cosi", bufs=2,
                                name=f"cosi_{c}")
                nc.vector.tensor_tensor(cosi[:], cos_sb[:, cs], inv128[:], ALU.mult)
                sini = sbs.tile([128, TOKC], MDT, tag="sini", bufs=2,
                                name=f"sini_{c}")
                nc.vector.tensor_tensor(sini[:], sin_sb[:, cs], inv128[:], ALU.mult)
                return cosi, sini, inv128

            def emit_inproj_m(c, m, state):
                cs = slice(c * TOKC, (c + 1) * TOKC)
                cosi, sini, inv128 = state
                ip = ps.tile([128, TOKC], f32, tag="acc", bufs=2,
                             name=f"ip{m}_{c}")
                for kt in range(NKT):
                    nc.tensor.matmul(ip[:], w_in_sb[:, kt, ts(m, 128)],
                                     xt_sb[:, kt, cs],
                                     start=(kt == 0), stop=(kt == NKT - 1))
                nc.vector.tensor_copy(qkv[:, m, cs], ip[:])
                if m < 5:
                    # rope in place, inv_rms folded into the tables.  The
                    # rotate-half partition swap runs as 4 small SBUF->SBUF
                    # DMAs (sin table is pre-rotated host-side), leaving
                    # only 3 full-width DVE ops.
                    rot = sbs.tile([128, TOKC], MDT, tag="rot", bufs=2,
                                   name=f"rot_{c}_{m}")
                    for dst, src in ((0, 32), (32, 0), (64, 96), (96, 64)):
                        nc.sync.dma_start(rot[dst:dst + 32, :],
                                          qkv[src:src + 32, m, cs])
                    nc.vector.tensor_tensor(rot[:], rot[:], sini[:], ALU.mult)
                    nc.vector.tensor_tensor(qkv[:, m, cs], qkv[:, m, cs],
                                            cosi[:], ALU.mult)
                    nc.vector.tensor_tensor(qkv[:, m, cs], qkv[:, m, cs],
                                            rot[:], ALU.add)
                else:
                    # V: fold per-token inv_rms once, then transpose to
                    # [token, dv] tiles
                    nc.vector.tensor_tensor(qkv[:, 5, cs], qkv[:, 5, cs],
                                            inv128[:], ALU.mult)
                    for tl in range(TOKC // 128):
                        t = c * (TOKC // 128) + tl
                        vtt = sbs.tile([128, 128], MDT, tag="vtt", bufs=2,
                                       name=f"vtt_{t}")
                        nc.sync.dma_start(vtt[:], qkv[:, 5, ts(t, 128)],
                                          transpose=True)
                        nc.vector.tensor_copy(vA[:, t, 0:64], vtt[:, 0:64])
                        nc.vector.tensor_copy(vB[:, t, 0:64], vtt[:, 64:128])

            def emit_attn_pair(c, p):
                cs = slice(c * TOKC, (c + 1) * TOKC)
                n_t = 4 * (c + 1)
                avA = ps.tile([65, TOKC], f32, tag="av", bufs=2,
                              name=f"avA_{c}_{p}")
                avB = ps.tile([65, TOKC], f32, tag="av", bufs=2,
                              name=f"avB_{c}_{p}")
                for t in range(n_t):
                    j0 = max(0, t - 4 * c) * 128
                    diag = t >= 4 * c
                    qk = ps.tile([128, 2, TOKC], f32, tag="qk", bufs=2,
                                 name=f"qk_{c}_{p}_{t}")
                    # the pair's two heads: row-tiled concurrent K=64 matmuls
                    nc.tensor.matmul(
                        qk[:, 0, j0:],
                        qkv[0:64, 4, ts(t, 128)],
                        qkv[0:64, p, c * TOKC + j0:(c + 1) * TOKC],
                        start=True, stop=not diag,
                    )
                    nc.tensor.matmul(
                        qk[:, 1, j0:],
                        qkv[64:128, 4, ts(t, 128)],
                        qkv[64:128, p, c * TOKC + j0:(c + 1) * TOKC],
                        start=True, stop=not diag,
                    )
                    if diag:
                        # causal mask for the diagonal 128x128 block as a PE
                        # accumulate: qk[k, j] += -30 * [j < k] (msk^T @ I),
                        # keeping the exp -> av chain off the vector engine.
                        nc.tensor.matmul(qk[:, 0, j0:j0 + 128], msk_sb[:],
                                         id_sb[:], start=False, stop=True)
                        nc.tensor.matmul(qk[:, 1, j0:j0 + 128], msk_sb[:],
                                         id_sb[:], start=False, stop=True)
                    e = sbs.tile([128, 2, TOKC], MDT, tag="e", bufs=4,
                                 name=f"e_{c}_{p}_{t}")
                    nc.scalar.activation(e[:, :, j0:], qk[:, :, j0:], AF.Exp)
                    nc.tensor.matmul(avA[:, j0:], vA[:, t, :], e[:, 0, j0:],
                                     start=(t == 0), stop=(t == n_t - 1))
                    nc.tensor.matmul(avB[:, j0:], vB[:, t, :], e[:, 1, j0:],
                                     start=(t == 0), stop=(t == n_t - 1))
                # softmax denominators: row 64 of each AV psum.  Copy the
                # rows to SBUF partition 0 first: reciprocal_approx_fast
                # (custom DVE op) reads the wrong partition when its PSUM
                # source has a non-zero base partition (HW-verified).
                dinvA = sbs.tile([1, TOKC], f32, tag="dinvA", bufs=2,
                                 name=f"dinvA_{c}_{p}")
                nc.vector.tensor_copy(dinvA[:], avA[64:65, :])
                nc.vector.reciprocal_approx_fast(dinvA[:], dinvA[:])
                dinvB = sbs.tile([1, TOKC], f32, tag="dinvB", bufs=2,
                                 name=f"dinvB_{c}_{p}")
                nc.vector.tensor_copy(dinvB[:], avB[64:65, :])
                nc.vector.reciprocal_approx_fast(dinvB[:], dinvB[:])
                nc.sync.dma_start(db_dr[c, p, 0][None, :], dinvA[:])
                nc.sync.dma_start(db_dr[c, p, 1][None, :], dinvB[:])
                dbA = sbs.tile([64, TOKC], f32, tag="dbA", bufs=2,
                               name=f"dbA_{c}_{p}")
                nc.sync.dma_start(
                    dbA[:], db_dr[c, p, 0][None, :].to_broadcast((64, TOKC)))
                dbB = sbs.tile([64, TOKC], f32, tag="dbB", bufs=2,
                               name=f"dbB_{c}_{p}")
                nc.sync.dma_start(
                    dbB[:], db_dr[c, p, 1][None, :].to_broadcast((64, TOKC)))
                nc.vector.tensor_tensor(oT[0:64, p, cs], avA[0:64, :],
                                        dbA[:], ALU.mult)
                nc.vector.tensor_tensor(oT[64:128, p, cs], avB[0:64, :],
                                        dbB[:], ALU.mult)

            def emit_outproj_part(c, ms):
                cs = slice(c * TOKC, (c + 1) * TOKC)
                for m in ms:
                    op = ps.tile([128, TOKC], f32, tag="acc", bufs=2,
                                 name=f"op_{c}_{m}")
                    for kt in range(4):
                        nc.tensor.matmul(op[:], w_out_sb[:, kt, ts(m, 128)],
                                         oT[:, kt, cs],
                                         start=(kt == 0), stop=(kt == 3))
                    yt = sbs.tile([128, TOKC], MDT, tag="yt", bufs=3,
                                  name=f"yt_{c}_{m}")
                    nc.vector.tensor_copy(yt[:], op[:])
                    nc.sync.dma_start(yT_d[ts(m, 128), cs], yt[:])

            for c in range(NCH):
                st = emit_prelude(c)
                for m in range(6):
                    emit_inproj_m(c, m, st)
                    if c > 0 and m < 4:
                        emit_attn_pair(c - 1, m)
                    if c > 0 and m >= 4:
                        emit_outproj_part(c - 1, range(8 * (m - 4), 8 * (m - 3)))
            for p in range(4):
                emit_attn_pair(NCH - 1, p)
            emit_outproj_part(NCH - 1, range(16))

    nc.finalize()
    return nc


# ------------------------------- host side ----------------------------------

def _rope_tables(S):
    inv_freq = ROPE_THETA ** (-np.arange(0, 64, 2, dtype=np.float64) / 64.0)  # [32]
    ang = np.arange(S, dtype=np.float64)[:, None] * inv_freq[None, :]          # [S, 32]
    cosb = np.cos(ang).T.astype(np.float32)   # [32, S]
    sinb = np.sin(ang).T.astype(np.float32)
    cos128 = np.tile(cosb, (4, 1))                                             # [128, S]
    # pre-rotated sign pattern: row block dst reads the sin factor of the
    # block it was swapped with on-device ((0,32),(32,0),(64,96),(96,64))
    sin128 = np.concatenate([-sinb, sinb, -sinb, sinb], axis=0)                # [128, S]
    return np.ascontiguousarray(cos128), np.ascontiguousarray(sin128)


def host_prepare(x, w_in, w_out, rms_w):
    """Build the 8 per-core input maps."""
    S = x.shape[1]
    x = np.asarray(x, dtype=np.float32)
    w_eff = np.asarray(w_in, dtype=np.float32) * np.asarray(rms_w, np.float32)[None, :]
    w_out = np.asarray(w_out, dtype=np.float32)
    cos128, sin128 = _rope_tables(S)
    mskA = np.ascontiguousarray(
        -30.0 * np.triu(np.ones((128, 128), dtype=np.float32), 1))
    id128 = np.eye(128, dtype=np.float32)
    qscale = np.float32(64 ** -0.5)

    in_maps = []
    for core in range(NCORES):
        b, j = divmod(core, 4)
        g0, g1 = 2 * j, 2 * j + 1
        rows = []
        for p in range(4):
            for g in (g0, g1):
                rows.extend(range((g * 4 + p) * 64, (g * 4 + p) * 64 + 64))
        for g in (g0, g1):
            rows.extend(range(2048 + g * 64, 2048 + g * 64 + 64))
        for g in (g0, g1):
            rows.extend(range(2560 + g * 64, 2560 + g * 64 + 64))
        w_slice = w_eff[rows, :].copy()
        w_slice[:512, :] *= qscale
        cols = []
        for p in range(4):
            for g in (g0, g1):
                cols.extend(range((g * 4 + p) * 64, (g * 4 + p) * 64 + 64))
        in_maps.append({
            "xT": np.ascontiguousarray(x[b].T).astype(MDT_NP),
            "w_inT": np.ascontiguousarray(w_slice.T).astype(MDT_NP),
            "w_outT": np.ascontiguousarray(w_out[:, cols].T).astype(MDT_NP),
            "cos_t": cos128.astype(MDT_NP),
            "sin_t": sin128.astype(MDT_NP),
            "mskA": mskA.astype(MDT_NP),
            "id128": id128.astype(MDT_NP),
            "oner": np.ones(1, dtype=MDT_NP),
            "epsc": np.full(1, RMS_EPS, dtype=np.float32),
        })
    return in_maps


def assemble(x, results):
    x = np.asarray(x, dtype=np.float32)
    b0 = (results[0]["yT"].astype(np.float32) + results[1]["yT"].astype(np.float32)
          + results[2]["yT"].astype(np.float32) + results[3]["yT"].astype(np.float32))
    b1 = (results[4]["yT"].astype(np.float32) + results[5]["yT"].astype(np.float32)
          + results[6]["yT"].astype(np.float32) + results[7]["yT"].astype(np.float32))
    out = np.empty_like(x)
    out[0] = x[0] + b0.T
    out[1] = x[1] + b1.T
    return out


_PROGRAMS = {}


def _get_program(S):
    if S not in _PROGRAMS:
        _PROGRAMS[S] = build_program(S)
    return _PROGRAMS[S]


def run(x, w_in, w_out, rms_w, trace=False):
    from concourse.bass_utils import run_bass_kernel_spmd
    nc = _get_program(x.shape[1])
    in_maps = host_prepare(x, w_in, w_out, rms_w)
    res = run_bass_kernel_spmd(nc, in_maps, list(range(NCORES)), trace=trace)
    return assemble(x, res.results), res


def kernel(x, w_in, w_out, rms_w):
    out, _ = run(np.asarray(x), np.asarray(w_in), np.asarray(w_out),
                 np.asarray(rms_w))
    return out


# revision 24
# speedup vs baseline: 1.1296x; 1.0947x over previous
"""GroupedQueryAttention Trainium2 kernel (8-core SPMD), v2.

Reference op: RMSNorm -> in-proj (q/k/v) -> RoPE -> causal GQA attention
-> out-proj -> residual.  b=2, s=2048, d_model=2048, 32 q-heads / 8 KV
groups, head dim 64, fp32.

Sharding: core c handles batch b = c//4 and KV groups (2j, 2j+1), j = c%4
(data parallel over batch x tensor parallel over KV groups, Megatron
style).  Each core computes the full in-projection restricted to its 8
heads' channels, attention for its 8 heads, and a partial out-projection
(row-parallel).  The host sums the 4 partials per batch and adds the
residual.

v2 changes vs v1 (993us):
  * qk PSUM double-buffered (2 tags x 2 banks) so QK(t+1) overlaps
    exp(t); PSUM = acc(ip/ss/op shared, 2) + qk(4) + av(2) = 8 banks.
  * inv_rms and softmax denominators via reciprocal_approx_fast on the
    natural [1/2, 512] rows - kills the DRAM transpose bounces and the
    ACT denominator copies.
  * All DVE traffic 16-bit where possible (cos/sin/tri tables f16) for
    DVE 2x mode; xsq moved to ACT (Square), V inv_rms scale folded as
    one TT instead of per-tile ACT scale-copies.
  * w_out and all of x resident in SBUF (no per-chunk weight reloads).
  * yT output f16 (halves store traffic; host accumulates in fp32).
"""

import numpy as np
from contextlib import ExitStack

import concourse.bass as bass
from concourse import bacc as _bacc
import concourse.mybir as mybir
import concourse.tile as tile
from concourse.bass import ts

import os
f32 = mybir.dt.float32
f32r = mybir.dt.float32r
f16 = mybir.dt.float16
MDT = {"f32r": f32r, "f16": f16, "bf16": mybir.dt.bfloat16}[os.environ.get("GQA_MM_DT", "f16")]
try:
    import ml_dtypes
    _BF16_NP = ml_dtypes.bfloat16
except ImportError:
    _BF16_NP = None
MDT_NP = {f32r: np.float32, f16: np.float16, mybir.dt.bfloat16: _BF16_NP}[MDT]
AF = mybir.ActivationFunctionType
ALU = mybir.AluOpType

D = 2048          # model dim
CH = 768          # per-core in-proj channels (8 q heads + 2 k + 2 v)
TOKC = 512        # token chunk
NKT = D // 128    # 16 k-tiles over model dim
RMS_EPS = 1e-6
ROPE_THETA = 10000.0
NCORES = 8


def build_program(S=2048):
    NCH = S // TOKC          # token chunks
    NSK = S // 128           # sk tiles
    nc = _bacc.Bacc(None)

    xT_d = nc.dram_tensor("xT", [D, S], MDT, kind="ExternalInput")
    w_inT_d = nc.dram_tensor("w_inT", [D, CH], MDT, kind="ExternalInput")
    w_outT_d = nc.dram_tensor("w_outT", [512, D], MDT, kind="ExternalInput")
    cos_d = nc.dram_tensor("cos_t", [128, S], MDT, kind="ExternalInput")
    sin_d = nc.dram_tensor("sin_t", [128, S], MDT, kind="ExternalInput")
    msk_d = nc.dram_tensor("mskA", [128, 128], MDT, kind="ExternalInput")
    id_d = nc.dram_tensor("id128", [128, 128], MDT, kind="ExternalInput")
    oner_d = nc.dram_tensor("oner", [1], MDT, kind="ExternalInput")
    eps_d = nc.dram_tensor("epsc", [1], f32, kind="ExternalInput")
    yT_d = nc.dram_tensor("yT", [D, S], MDT, kind="ExternalOutput")

    with tile.TileContext(nc) as tc, ExitStack() as ctx:
        sb = ctx.enter_context(tc.tile_pool(name="sb", bufs=1))
        sbs = ctx.enter_context(tc.tile_pool(name="sbs", bufs=2))
        dramp = ctx.enter_context(tc.tile_pool(name="dram", bufs=1, space="DRAM"))

        # persistent SBUF
        w_in_sb = sb.tile([128, NKT, CH], MDT, name="w_in_sb")
        w_out_sb = sb.tile([128, 4, D], MDT, name="w_out_sb")
        xt_sb = sb.tile([128, NKT, S], MDT, name="xt_sb")
        qkv = sb.tile([128, 6, S], MDT, name="qkv")        # ch tiles 0-3 q pairs, 4 k, 5 v
        oT = sb.tile([128, 4, S], MDT, name="oT")
        vA = sb.tile([128, NSK, 65], MDT, name="vA")       # V + ones col, group 0
        vB = sb.tile([128, NSK, 65], MDT, name="vB")       # group 1
        cos_sb = sb.tile([128, S], MDT, name="cos_sb")
        sin_sb = sb.tile([128, S], MDT, name="sin_sb")
        msk_sb = sb.tile([128, 128], MDT, name="msk_sb")
        id_sb = sb.tile([128, 128], MDT, name="id_sb")
        ones_sb = sb.tile([128, 1], MDT, name="ones_sb")
        eps_sb = sb.tile([1, 1], f32, name="eps_sb")

        nrm_dr = dramp.tile([NCH, TOKC], MDT, name="nrm_dr")
        db_dr = dramp.tile([NCH, 4, 2, TOKC], f32, name="db_dr")

        w_inT_v = w_inT_d.rearrange("(o p) c -> p o c", p=128)
        nc.sync.dma_start(w_in_sb[:], w_inT_v[:])
        w_outT_v = w_outT_d.rearrange("(o p) c -> p o c", p=128)
        nc.sync.dma_start(w_out_sb[:], w_outT_v[:])
        xT_v = xT_d.rearrange("(o p) s -> p o s", p=128)
        nc.sync.dma_start(cos_sb[:], cos_d[:])
        nc.sync.dma_start(sin_sb[:], sin_d[:])
        nc.sync.dma_start(msk_sb[:], msk_d[:])
        nc.sync.dma_start(id_sb[:], id_d[:])
        nc.sync.dma_start(ones_sb[:], oner_d[None, :].to_broadcast((128, 1)))
        nc.sync.dma_start(vA[:, :, 64:65], oner_d[None, None, :].to_broadcast((128, NSK, 1)))
        nc.sync.dma_start(vB[:, :, 64:65], oner_d[None, None, :].to_broadcast((128, NSK, 1)))
        nc.sync.dma_start(eps_sb[:], eps_d[None, :])

        # PSUM: acc (in-proj / sum-sq / out-proj, 1 bank x2) + qk (2 banks
        # x2) + av (1 bank x2) = 8 banks.
        with tc.tile_pool(name="ps", bufs=1, space="PSUM") as ps:

            def emit_prelude(c):
                cs = slice(c * TOKC, (c + 1) * TOKC)
                nc.sync.dma_start(xt_sb[:, :, cs], xT_v[:, :, cs])
                # sum of squares -> 1/rms row, then broadcast
                ss = ps.tile([1, TOKC], f32, tag="acc", bufs=2, name=f"ss_{c}")
                for kt in range(NKT):
                    xsq = sbs.tile([128, TOKC], MDT, tag="xsq", bufs=3,
                                   name=f"xsq_{c}_{kt}")
                    nc.vector.tensor_tensor(xsq[:], xt_sb[:, kt, cs],
                                            xt_sb[:, kt, cs], ALU.mult)
                    nc.tensor.matmul(ss[:], ones_sb[:], xsq[:],
                                     start=(kt == 0), stop=(kt == NKT - 1))
                # 1/rms = exp(-0.5 * ln(ss/D + eps)): ln+exp live in one ACT
                # table set (no sqrt-set thrash against attention exp).
                ln_row = sbs.tile([1, TOKC], f32, tag="ln_row", bufs=2,
                                  name=f"ln_row_{c}")
                nc.scalar.activation(ln_row[:], ss[:], AF.Ln,
                                     bias=eps_sb[:], scale=1.0 / D)
                inv16 = sbs.tile([1, TOKC], MDT, tag="inv16", bufs=2,
                                 name=f"inv16_{c}")
                nc.scalar.activation(inv16[:], ln_row[:], AF.Exp, scale=-0.5)
                nc.sync.dma_start(nrm_dr[c][None, :], inv16[:])
                inv128 = sbs.tile([128, TOKC], MDT, tag="inv128", bufs=2,
                                  name=f"inv128_{c}")
                nc.sync.dma_start(inv128[:],
                                  nrm_dr[c][None, :].to_broadcast((128, TOKC)))
                # rope tables scaled by inv_rms (f16 for DVE 2x mode)
                cosi = sbs.tile([128, TOKC], MDT, tag="cosi", bufs=2,
                                name=f"cosi_{c}")
                nc.vector.tensor_tensor(cosi[:], cos_sb[:, cs], inv128[:], ALU.mult)
                sini = sbs.tile([128, TOKC], MDT, tag="sini", bufs=2,
                                name=f"sini_{c}")
                nc.vector.tensor_tensor(sini[:], sin_sb[:, cs], inv128[:], ALU.mult)
                return cosi, sini, inv128

            def emit_inproj_m(c, m, state):
                cs = slice(c * TOKC, (c + 1) * TOKC)
                cosi, sini, inv128 = state
                ip = ps.tile([128, TOKC], f32, tag="acc", bufs=2,
                             name=f"ip{m}_{c}")
                for kt in range(NKT):
                    nc.tensor.matmul(ip[:], w_in_sb[:, kt, ts(m, 128)],
                                     xt_sb[:, kt, cs],
                                     start=(kt == 0), stop=(kt == NKT - 1))
                nc.vector.tensor_copy(qkv[:, m, cs], ip[:])
                if m < 5:
                    # rope in place, inv_rms folded into the tables.  The
                    # rotate-half partition swap runs as 4 small SBUF->SBUF
                    # DMAs (sin table is pre-rotated host-side), leaving
                    # only 3 full-width DVE ops.
                    rot = sbs.tile([128, TOKC], MDT, tag="rot", bufs=2,
                                   name=f"rot_{c}_{m}")
                    for dst, src in ((0, 32), (32, 0), (64, 96), (96, 64)):
                        nc.sync.dma_start(rot[dst:dst + 32, :],
                                          qkv[src:src + 32, m, cs])
                    nc.vector.tensor_tensor(rot[:], rot[:], sini[:], ALU.mult)
                    nc.vector.tensor_tensor(qkv[:, m, cs], qkv[:, m, cs],
                                            cosi[:], ALU.mult)
                    nc.vector.tensor_tensor(qkv[:, m, cs], qkv[:, m, cs],
                                            rot[:], ALU.add)
                else:
                    # V: fold per-token inv_rms once, then transpose to
                    # [token, dv] tiles
                    nc.vector.tensor_tensor(qkv[:, 5, cs], qkv[:, 5, cs],
                                            inv128[:], ALU.mult)
                    for tl in range(TOKC // 128):
                        t = c * (TOKC // 128) + tl
                        vtt = sbs.tile([128, 128], MDT, tag="vtt", bufs=2,
                                       name=f"vtt_{t}")
                        nc.sync.dma_start(vtt[:], qkv[:, 5, ts(t, 128)],
                                          transpose=True)
                        nc.vector.tensor_copy(vA[:, t, 0:64], vtt[:, 0:64])
                        nc.vector.tensor_copy(vB[:, t, 0:64], vtt[:, 64:128])

            def emit_attn_pair(c, p):
                cs = slice(c * TOKC, (c + 1) * TOKC)
                n_t = 4 * (c + 1)
                avA = ps.tile([65, TOKC], f32, tag="av", bufs=2,
                              name=f"avA_{c}_{p}")
                avB = ps.tile([65, TOKC], f32, tag="av", bufs=2,
                              name=f"avB_{c}_{p}")
                def emit_av(pe, pj0, pt):
                    nc.tensor.matmul(avA[:, pj0:], vA[:, pt, :], pe[:, 0, pj0:],
                                     start=(pt == 0), stop=(pt == n_t - 1))
                    nc.tensor.matmul(avB[:, pj0:], vB[:, pt, :], pe[:, 1, pj0:],
                                     start=(pt == 0), stop=(pt == n_t - 1))

                pend = None  # software-pipeline: av(t-1) emitted after QK(t)
                for t in range(n_t):
                    j0 = max(0, t - 4 * c) * 128
                    diag = t >= 4 * c
                    qk = ps.tile([128, 2, TOKC], f32, tag="qk", bufs=2,
                                 name=f"qk_{c}_{p}_{t}")
                    # the pair's two heads: row-tiled concurrent K=64 matmuls
                    nc.tensor.matmul(
                        qk[:, 0, j0:],
                        qkv[0:64, 4, ts(t, 128)],
                        qkv[0:64, p, c * TOKC + j0:(c + 1) * TOKC],
                        start=True, stop=not diag,
                    )
                    nc.tensor.matmul(
                        qk[:, 1, j0:],
                        qkv[64:128, 4, ts(t, 128)],
                        qkv[64:128, p, c * TOKC + j0:(c + 1) * TOKC],
                        start=True, stop=not diag,
                    )
                    if diag:
                        # causal mask for the diagonal 128x128 block as a PE
                        # accumulate: qk[k, j] += -30 * [j < k] (msk^T @ I),
                        # keeping the exp -> av chain off the vector engine.
                        nc.tensor.matmul(qk[:, 0, j0:j0 + 128], msk_sb[:],
                                         id_sb[:], start=False, stop=True)
                        nc.tensor.matmul(qk[:, 1, j0:j0 + 128], msk_sb[:],
                                         id_sb[:], start=False, stop=True)
                    e = sbs.tile([128, 2, TOKC], MDT, tag="e", bufs=4,
                                 name=f"e_{c}_{p}_{t}")
                    nc.scalar.activation(e[:, :, j0:], qk[:, :, j0:], AF.Exp)
                    if pend is not None:
                        emit_av(*pend)
                    pend = (e, j0, t)
                emit_av(*pend)
                # softmax denominators: row 64 of each AV psum.  Copy the
                # rows to SBUF partition 0 first: reciprocal_approx_fast
                # (custom DVE op) reads the wrong partition when its PSUM
                # source has a non-zero base partition (HW-verified).
                dinvA = sbs.tile([1, TOKC], f32, tag="dinvA", bufs=2,
                                 name=f"dinvA_{c}_{p}")
                nc.vector.tensor_copy(dinvA[:], avA[64:65, :])
                nc.vector.reciprocal_approx_fast(dinvA[:], dinvA[:])
                dinvB = sbs.tile([1, TOKC], f32, tag="dinvB", bufs=2,
                                 name=f"dinvB_{c}_{p}")
                nc.vector.tensor_copy(dinvB[:], avB[64:65, :])
                nc.vector.reciprocal_approx_fast(dinvB[:], dinvB[:])
                nc.sync.dma_start(db_dr[c, p, 0][None, :], dinvA[:])
                nc.sync.dma_start(db_dr[c, p, 1][None, :], dinvB[:])
                dbA = sbs.tile([64, TOKC], f32, tag="dbA", bufs=2,
                               name=f"dbA_{c}_{p}")
                nc.sync.dma_start(
                    dbA[:], db_dr[c, p, 0][None, :].to_broadcast((64, TOKC)))
                dbB = sbs.tile([64, TOKC], f32, tag="dbB", bufs=2,
                               name=f"dbB_{c}_{p}")
                nc.sync.dma_start(
                    dbB[:], db_dr[c, p, 1][None, :].to_broadcast((64, TOKC)))
                nc.vector.tensor_tensor(oT[0:64, p, cs], avA[0:64, :],
                                        dbA[:], ALU.mult)
                nc.vector.tensor_tensor(oT[64:128, p, cs], avB[0:64, :],
                                        dbB[:], ALU.mult)

            def emit_outproj_part(c, ms):
                cs = slice(c * TOKC, (c + 1) * TOKC)
                for m in ms:
                    op = ps.tile([128, TOKC], f32, tag="acc", bufs=2,
                                 name=f"op_{c}_{m}")
                    for kt in range(4):
                        nc.tensor.matmul(op[:], w_out_sb[:, kt, ts(m, 128)],
                                         oT[:, kt, cs],
                                         start=(kt == 0), stop=(kt == 3))
                    yt = sbs.tile([128, TOKC], MDT, tag="yt", bufs=3,
                                  name=f"yt_{c}_{m}")
                    nc.vector.tensor_copy(yt[:], op[:])
                    nc.sync.dma_start(yT_d[ts(m, 128), cs], yt[:])

            for c in range(NCH):
                st = emit_prelude(c)
                for m in range(6):
                    emit_inproj_m(c, m, st)
                    if c > 0 and m < 4:
                        emit_attn_pair(c - 1, m)
                    if c > 0 and m >= 4:
                        emit_outproj_part(c - 1, range(8 * (m - 4), 8 * (m - 3)))
            for p in range(4):
                emit_attn_pair(NCH - 1, p)
            emit_outproj_part(NCH - 1, range(16))

    nc.finalize()
    return nc


# ------------------------------- host side ----------------------------------

def _rope_tables(S):
    inv_freq = ROPE_THETA ** (-np.arange(0, 64, 2, dtype=np.float64) / 64.0)  # [32]
    ang = np.arange(S, dtype=np.float64)[:, None] * inv_freq[None, :]          # [S, 32]
    cosb = np.cos(ang).T.astype(np.float32)   # [32, S]
    sinb = np.sin(ang).T.astype(np.float32)
    cos128 = np.tile(cosb, (4, 1))                                             # [128, S]
    # pre-rotated sign pattern: row block dst reads the sin factor of the
    # block it was swapped with on-device ((0,32),(32,0),(64,96),(96,64))
    sin128 = np.concatenate([-sinb, sinb, -sinb, sinb], axis=0)                # [128, S]
    return np.ascontiguousarray(cos128), np.ascontiguousarray(sin128)


def host_prepare(x, w_in, w_out, rms_w):
    """Build the 8 per-core input maps."""
    S = x.shape[1]
    x = np.asarray(x, dtype=np.float32)
    w_eff = np.asarray(w_in, dtype=np.float32) * np.asarray(rms_w, np.float32)[None, :]
    w_out = np.asarray(w_out, dtype=np.float32)
    cos128, sin128 = _rope_tables(S)
    mskA = np.ascontiguousarray(
        -30.0 * np.triu(np.ones((128, 128), dtype=np.float32), 1))
    id128 = np.eye(128, dtype=np.float32)
    qscale = np.float32(64 ** -0.5)

    in_maps = []
    for core in range(NCORES):
        b, j = divmod(core, 4)
        g0, g1 = 2 * j, 2 * j + 1
        rows = []
        for p in range(4):
            for g in (g0, g1):
                rows.extend(range((g * 4 + p) * 64, (g * 4 + p) * 64 + 64))
        for g in (g0, g1):
            rows.extend(range(2048 + g * 64, 2048 + g * 64 + 64))
        for g in (g0, g1):
            rows.extend(range(2560 + g * 64, 2560 + g * 64 + 64))
        w_slice = w_eff[rows, :].copy()
        w_slice[:512, :] *= qscale
        cols = []
        for p in range(4):
            for g in (g0, g1):
                cols.extend(range((g * 4 + p) * 64, (g * 4 + p) * 64 + 64))
        in_maps.append({
            "xT": np.ascontiguousarray(x[b].T).astype(MDT_NP),
            "w_inT": np.ascontiguousarray(w_slice.T).astype(MDT_NP),
            "w_outT": np.ascontiguousarray(w_out[:, cols].T).astype(MDT_NP),
            "cos_t": cos128.astype(MDT_NP),
            "sin_t": sin128.astype(MDT_NP),
            "mskA": mskA.astype(MDT_NP),
            "id128": id128.astype(MDT_NP),
            "oner": np.ones(1, dtype=MDT_NP),
            "epsc": np.full(1, RMS_EPS, dtype=np.float32),
        })
    return in_maps


def assemble(x, results):
    x = np.asarray(x, dtype=np.float32)
    b0 = (results[0]["yT"].astype(np.float32) + results[1]["yT"].astype(np.float32)
          + results[2]["yT"].astype(np.float32) + results[3]["yT"].astype(np.float32))
    b1 = (results[4]["yT"].astype(np.float32) + results[5]["yT"].astype(np.float32)
          + results[6]["yT"].astype(np.float32) + results[7]["yT"].astype(np.float32))
    out = np.empty_like(x)
    out[0] = x[0] + b0.T
    out[1] = x[1] + b1.T
    return out


_PROGRAMS = {}


def _get_program(S):
    if S not in _PROGRAMS:
        _PROGRAMS[S] = build_program(S)
    return _PROGRAMS[S]


def run(x, w_in, w_out, rms_w, trace=False):
    from concourse.bass_utils import run_bass_kernel_spmd
    nc = _get_program(x.shape[1])
    in_maps = host_prepare(x, w_in, w_out, rms_w)
    res = run_bass_kernel_spmd(nc, in_maps, list(range(NCORES)), trace=trace)
    return assemble(x, res.results), res


def kernel(x, w_in, w_out, rms_w):
    out, _ = run(np.asarray(x), np.asarray(w_in), np.asarray(w_out),
                 np.asarray(rms_w))
    return out
